# revision 11
# baseline (speedup 1.0000x reference)
"""Causal self-attention (B=2, T=2048, C=1024, H=16) on 8 Trainium2 cores.

Sharding: tensor-parallel over heads (2 heads/core). Each core computes the
QKV projection for its heads, causal attention, and a partial c_proj output;
partials (and b_proj) are summed on the host.

v3 dataflow — natural-orientation AV (every matmul has a full 128-partition
output and near-minimal cost-model free size), with software pipelining
tuned for the in-order per-engine instruction streams:
  xT fp16 [C, B*T]                       (host pre-transposes x)
  qT/kT [128, B*T] fp16 = Wqk^T @ x + b  (PE, 512-free tiles)
  v_nat [k 128, 65] fp16 = x^T @ Wv | 1  (PE, 64-free tiles, bias via rank-1)
  S^T [k 128, q 512] f32 = K Q^T         (PE; causal tiles only, diag-trimmed)
  et = exp(S^T/8) fp16                   (ACT; tri-mask on diag via Pool)
  yp [q 128, 65] f32 += et_j^T @ v_nat   (PE; col 64 accumulates softmax sums)
  y = yp[:,0:64] * recip(yp[:,64]) f32r  (DVE per-partition scalars, no bcast)
  ynT [64, 128] = PE-transpose(y)        (f32r)
  out^T [c 128, rows] f32 = Wp^T @ ynT   (PE, f32r) -> fp16 staging -> DRAM

Scheduling: AV bursts are delayed 3 S-tiles behind their exp, transposes 2
bursts behind their normalize, and projection/c_proj work is drip-fed from a
work queue (one thunk per S-tile) so the in-order PE stream never waits on a
cross-engine semaphore chain. PSUM: 4 sps + 1 yp/transpose + 2 qk/cproj +
1 v = 8 banks exactly.
"""

from collections import deque

import numpy as np

import concourse.bass as bass
import concourse.tile as tile
from concourse import bacc, mybir
from concourse.bass_utils import run_bass_kernel_spmd
from concourse.masks import make_identity

F32 = mybir.dt.float32
F32R = mybir.dt.float32r
F16 = mybir.dt.float16

B, T, C, H = 2, 2048, 1024, 16
HS = C // H            # 64 head dim
NCORES = 8
HL = H // NCORES       # 2 local heads
LC = HL * HS           # 128 local q (or k, or v) channels
R = B * T              # 4096 rows
KC = C // 128          # 8 contraction chunks for projections
RT = 512               # row tile for qk-proj / c_proj
NRT = R // RT          # 8
QW = 512               # attention q window (S/exp tile width)
NQW = T // QW          # 4 per batch
KA = 128               # attention k chunk (partition dim)
NCC = C // 128         # 8 c_proj output chunks

AV_DELAY = 3           # S-tiles between exp issue and its AV burst
TP_DELAY = 2           # bursts between normalize and its PE transpose


def build_program():
    nc = bacc.Bacc("TRN2", target_bir_lowering=False, debug=False,
                   num_devices=NCORES)

    xT = nc.dram_tensor("xT", [C, R], F16, kind="ExternalInput").ap()
    wqk = nc.dram_tensor("wqk", [C, 2 * LC], F16, kind="ExternalInput").ap()
    wv = nc.dram_tensor("wv", [C, HL, HS], F16, kind="ExternalInput").ap()
    wp = nc.dram_tensor("wp", [LC, C], F32R, kind="ExternalInput").ap()
    bqk = nc.dram_tensor("bqk", [2 * LC], F32, kind="ExternalInput").ap()
    bv = nc.dram_tensor("bv", [1, LC], F16, kind="ExternalInput").ap()
    trimask = nc.dram_tensor("trimask", [KA, KA], F16, kind="ExternalInput").ap()
    outT = nc.dram_tensor("outT", [C, R], F16, kind="ExternalOutput").ap()

    with tile.TileContext(nc) as tc:
        with (
            tc.tile_pool(name="consts", bufs=1) as consts,
            tc.tile_pool(name="weights", bufs=1) as weights,
            tc.tile_pool(name="big", bufs=1) as big,
            tc.tile_pool(name="xs", bufs=3) as xs_pool,
            tc.tile_pool(name="et", bufs=26) as et_pool,
            tc.tile_pool(name="ysb", bufs=4) as ysb_pool,
            tc.tile_pool(name="rec", bufs=4) as rec_pool,
            tc.tile_pool(name="osb", bufs=2) as osb_pool,
            tc.tile_pool(name="sps", bufs=4, space="PSUM") as sps_pool,
            tc.tile_pool(name="ypb", bufs=1, space="PSUM") as ypb_pool,
            tc.tile_pool(name="qkps", bufs=2, space="PSUM") as qkps_pool,
            tc.tile_pool(name="vps", bufs=1, space="PSUM") as vps_pool,
        ):
            lp = nc.allow_low_precision(
                reason="fp16/f32r attention pipeline; ~1e-3 rel err validated")
            lp.__enter__()

            # ---- constants ----
            identity = consts.tile([128, 128], F32R)
            make_identity(nc, identity)
            ones1 = consts.tile([1, 128], F16)
            nc.vector.memset(ones1, 1.0)
            tri_sb = consts.tile([KA, KA], F16)
            bqk_sb = consts.tile([128, 2], F32)
            bv_sb = consts.tile([1, LC], F16)

            # ---- weights ----
            wqk_sb = weights.tile([128, KC, 2 * LC], F16)
            wqk_r = wqk.rearrange("(kc p) n -> p kc n", p=128)
            wv_sb = weights.tile([128, KC, HL, HS], F16)
            wv_r = wv.rearrange("(kc p) h n -> p kc h n", p=128)
            wp_sb = weights.tile([LC, C], F32R)

            def load_consts():
                nc.sync.dma_start(out=tri_sb, in_=trimask)
                nc.sync.dma_start(
                    out=bqk_sb, in_=bqk.rearrange("(j p) -> p j", p=128))
                nc.sync.dma_start(out=bv_sb, in_=bv)
                nc.sync.dma_start(out=wv_sb, in_=wv_r)
                nc.sync.dma_start(out=wp_sb, in_=wp)

            # ---- persistent activations ----
            qT_s = big.tile([LC, R], F16, tag="qT")
            kT_s = big.tile([LC, R], F16, tag="kT")
            ynT_s = big.tile([LC, R], F32R, tag="ynT")
            # v_nat[:, b, h, kci, 0:64] = v rows; col 64 = 1.0 (softmax sums)
            v_nat = big.tile([KA, B, HL, T // KA, HS + 1], F16, tag="vnat")

            # persistent PSUM slot tiles (manual slot rotation keeps WAR
            # hazards per-slot instead of per-tile)
            ypb = ypb_pool.tile([128, 512], F32, tag="ypb")  # 3 yp + 2 tp
            vpsb = vps_pool.tile([128, 6 * HS], F32, tag="vpsb")  # 6 v slots

            def qkv_load(rt):
                x_sb = xs_pool.tile([128, KC, RT], F16, tag="xs",
                                    name=f"x_sb_rt{rt}")
                x_r = xT[:, rt * RT:(rt + 1) * RT].rearrange(
                    "(kc p) r -> p kc r", p=128)
                if rt == 0:
                    nc.sync.dma_start(out=x_sb[:, 0:1], in_=x_r[:, 0:1])
                    nc.sync.dma_start(out=x_sb[:, 1:KC], in_=x_r[:, 1:KC])
                else:
                    nc.sync.dma_start(out=x_sb, in_=x_r)
                return x_sb

            def qk_col(rt, x_sb, col, kc_outer_mate=None):
                """one projection column (q or k) for row tile rt."""
                span = slice(rt * RT, (rt + 1) * RT)
                dst = qT_s if col == 0 else kT_s
                ps = qkps_pool.tile([128, RT], F32, tag="qkps",
                                    name=f"qk_ps_rt{rt}c{col}")
                for kc in range(KC):
                    nc.tensor.matmul(
                        ps,
                        wqk_sb[:, kc, col * LC:(col + 1) * LC],
                        x_sb[:, kc, :],
                        start=(kc == 0), stop=(kc == KC - 1),
                    )
                nc.vector.tensor_scalar_add(
                    dst[:, span], ps, bqk_sb[:, col:col + 1])

            _vslot = [0]

            def v_half(rt, x_sb, h, chs):
                """v projection (natural layout) for 2 row chunks."""
                b = (rt * RT) // T
                for ch in chs:
                    kci = (rt * RT - b * T) // KA + ch
                    s = _vslot[0]
                    _vslot[0] = (s + 1) % 6
                    slot = vpsb[:, s * HS:(s + 1) * HS]
                    nc.tensor.matmul(
                        slot, ones1, bv_sb[:, h * HS:(h + 1) * HS],
                        start=True, stop=False)
                    for kc in range(KC):
                        nc.tensor.matmul(
                            slot,
                            x_sb[:, kc, ch * KA:(ch + 1) * KA],
                            wv_sb[:, kc, h, :],
                            start=False, stop=(kc == KC - 1),
                        )
                    nc.vector.tensor_copy(v_nat[:, b, h, kci, 0:HS], slot)

            def qkv_thunks(rt):
                """work-queue thunks for one qkv row tile."""
                state = {}

                def load():
                    state["x"] = qkv_load(rt)

                return [
                    lambda: (load(), qk_col(rt, state["x"], 0)),
                    lambda: qk_col(rt, state["x"], 1),
                    lambda: v_half(rt, state["x"], 0, (0, 1)),
                    lambda: v_half(rt, state["x"], 0, (2, 3)),
                    lambda: v_half(rt, state["x"], 1, (0, 1)),
                    lambda: v_half(rt, state["x"], 1, (2, 3)),
                ]

            _ccsplit = [0]

            def cproj_thunks(rt, lo, hi, tail=False):
                """work-queue thunks for a c_proj row range (one per cc)."""
                w = hi - lo
                state = {}

                def get_osb():
                    if "o" not in state:
                        state["o"] = osb_pool.tile(
                            [128, NCC, RT], F16, tag="osb",
                            name=f"o_sb_rt{rt}_{lo}")
                    return state["o"]

                def cc_piece(cc):
                    o_sb = get_osb()
                    pps = qkps_pool.tile([128, RT], F32, tag="qkps",
                                         name=f"pps_rt{rt}_{lo}c{cc}")
                    nc.tensor.matmul(
                        pps[:, 0:w],
                        wp_sb[:, cc * 128:(cc + 1) * 128],
                        ynT_s[:, rt * RT + lo: rt * RT + hi],
                        start=True, stop=True,
                    )
                    if tail:
                        nc.vector.tensor_copy(
                            o_sb[:, cc, 0:w // 2], pps[:, 0:w // 2])
                        nc.gpsimd.tensor_copy(
                            o_sb[:, cc, w // 2:w], pps[:, w // 2:w])
                        return
                    # ~28/64 copies on DVE, rest on Pool (load balance)
                    s = _ccsplit[0]
                    _ccsplit[0] = (s + 1) % 16
                    if s % 16 < 7:
                        nc.vector.tensor_copy(o_sb[:, cc, 0:w], pps[:, 0:w])
                    else:
                        nc.gpsimd.tensor_copy(o_sb[:, cc, 0:w], pps[:, 0:w])

                def dma_out():
                    o_sb = get_osb()
                    nc.sync.dma_start(
                        out=outT.rearrange("(cc p) r -> p cc r", p=128)[
                            :, :, rt * RT + lo: rt * RT + hi],
                        in_=o_sb[:, :, 0:w],
                    )

                return [(lambda cc=cc: cc_piece(cc)) for cc in range(NCC)] + \
                    [dma_out]

            # ---- attention machinery ----
            gtile = [0]          # S tiles emitted
            nburst = [0]         # AV bursts fired
            pending_b = deque()  # (gtile at creation, fire closure)
            pending_t = deque()  # (nburst at creation, fire closure)
            _yslot = [0]
            _tslot = [0]

            def make_tp(b, h, j, y_sb):
                def fire():
                    ts = _tslot[0]
                    _tslot[0] ^= 1
                    tdst = ypb[0:HS, 3 * (HS + 1) + ts * 128:
                               3 * (HS + 1) + (ts + 1) * 128].bitcast(F32R)
                    nc.tensor.transpose(tdst, y_sb, identity)
                    hsl = slice(h * HS, (h + 1) * HS)
                    base = b * T
                    dst = ynT_s[hsl, base + j * KA: base + (j + 1) * KA]
                    if j % 2 == 0:
                        nc.vector.tensor_copy(dst, tdst)
                    else:
                        nc.gpsimd.tensor_copy(dst, tdst)
                return fire

            def make_burst(b, h, q0, j, ets):
                def fire():
                    ys = _yslot[0]
                    _yslot[0] = (ys + 1) % 3
                    yp = ypb[:, ys * (HS + 1):(ys + 1) * (HS + 1)]
                    jw = j * KA - q0
                    for k2 in range(j + 1):
                        et = ets[k2]
                        nc.tensor.matmul(
                            yp,
                            et[:, jw:jw + KA],
                            v_nat[:, b, h, k2, :],
                            start=(k2 == 0), stop=(k2 == j),
                        )
                    rec = rec_pool.tile([128, 1], F32, tag="rec",
                                        name=f"rec_b{b}h{h}j{j}")
                    nc.vector.reciprocal(rec, yp[:, HS:HS + 1])
                    y_sb = ysb_pool.tile([128, HS], F32R, tag="ysb",
                                         name=f"y_b{b}h{h}j{j}")
                    nc.vector.tensor_scalar_mul(y_sb, yp[:, 0:HS], rec)
                    pending_t.append((nburst[0], make_tp(b, h, j, y_sb)))
                    nburst[0] += 1
                return fire

            def flush_pending(force_b=False, force_t=False):
                while pending_b and (force_b or
                                     gtile[0] - pending_b[0][0] >= AV_DELAY):
                    pending_b.popleft()[2]()
                while pending_t and (force_t or
                                     nburst[0] - pending_t[0][0] >= TP_DELAY):
                    pending_t.popleft()[1]()

            # ---- work queue ----
            workq = deque()  # (ready (widx,kc) | None, deadline widx | None, fn)

            def pop_work(widx, kc, budget=1):
                n = 0
                while workq and n < budget:
                    ready, _, fn = workq[0]
                    if ready is not None and (widx, kc) < ready:
                        break
                    workq.popleft()
                    fn()
                    n += 1

            def force_deadlines(widx):
                while workq:
                    _, deadline, fn = workq[0]
                    if deadline is None or deadline > widx:
                        break
                    workq.popleft()
                    fn()

            # ---- attention window ----
            def attn_window(widx, b, h, qw):
                base = b * T
                hsl = slice(h * HS, (h + 1) * HS)
                q0 = qw * QW
                nkc = (qw + 1) * (QW // KA)
                ets = []
                for kc in range(nkc):
                    diag = (kc * KA >= q0)
                    off = kc * KA - q0 if diag else 0
                    sps = sps_pool.tile([KA, QW], F32, tag="sps",
                                        name=f"sps_b{b}h{h}q{qw}k{kc}")
                    nc.tensor.matmul(
                        sps[:, off:QW],
                        kT_s[hsl, base + kc * KA: base + (kc + 1) * KA],
                        qT_s[hsl, base + q0 + off: base + q0 + QW],
                        start=True, stop=True,
                    )
                    gtile[0] += 1
                    et = et_pool.tile([KA, QW], F16, tag="et",
                                      name=f"et_b{b}h{h}q{qw}k{kc}")
                    nc.scalar.activation(
                        et[:, off:QW], sps[:, off:QW],
                        mybir.ActivationFunctionType.Exp,
                        scale=1.0 / np.sqrt(HS).item(),
                    )
                    if diag:
                        nc.gpsimd.tensor_mul(
                            et[:, off:off + KA], et[:, off:off + KA], tri_sb)
                        pending_b.append(
                            (gtile[0], kc, make_burst(b, h, q0, kc, ets)))
                    ets.append(et)
                    flush_pending()
                    pop_work(widx, kc)

            # ---- schedule ----
            x0 = qkv_load(0)
            nc.sync.dma_start(out=wqk_sb[:, 0:2], in_=wqk_r[:, 0:2])
            nc.sync.dma_start(out=wqk_sb[:, 2:KC], in_=wqk_r[:, 2:KC])
            load_consts()
            nc.vector.memset(v_nat[:, :, :, :, HS:HS + 1], 1.0)
            # rt0 inline, kc-outer-ish: q then k then v (x0 kc0 lands first)
            qk_col(0, x0, 0)
            qk_col(0, x0, 1)
            v_half(0, x0, 0, (0, 1))
            v_half(0, x0, 0, (2, 3))
            v_half(0, x0, 1, (0, 1))
            v_half(0, x0, 1, (2, 3))

            # queue the rest of the projection + c_proj work
            for rt in range(1, NRT):
                b = rt // NQW
                deadline = b * 8 + (rt % NQW)  # window (b, h0, qw=rt%4)
                for fn in qkv_thunks(rt):
                    workq.append((None, deadline, fn))
            for rt in range(NRT):
                b = rt // NQW
                qw = rt % NQW
                if rt == NRT - 1:
                    continue  # final row tile handled in the tail
                widx_done = b * 8 + 4 + qw  # window (b, h1, qw)
                ready = (widx_done + 1, 6)
                for fn in cproj_thunks(rt, 0, RT):
                    workq.append((ready, None, fn))

            windows = [(b, h, qw) for b in range(B) for h in range(HL)
                       for qw in range(NQW)]
            for widx, (b, h, qw) in enumerate(windows):
                force_deadlines(widx)
                attn_window(widx, b, h, qw)

            # ---- tail: drain pipeline + final row tile, fine-grained ----
            pop_work(10 ** 6, 0, budget=len(workq) + 1)
            # rows 3584:3840 need tps through j=13; rows 3840:4096 need j=15
            while pending_b and pending_b[0][1] <= 13:
                pending_b.popleft()[2]()
            flush_pending(force_t=True)
            for fn in cproj_thunks(NRT - 1, 0, 256):
                fn()
            while pending_b:
                pending_b.popleft()[2]()
            flush_pending(force_t=True)
            for fn in cproj_thunks(NRT - 1, 256, RT, tail=True):
                fn()

            lp.__exit__(None, None, None)

    nc.compile()
    return nc


_NC = None


def _get_nc():
    global _NC
    if _NC is None:
        _NC = build_program()
    return _NC


def make_in_maps(x, W_attn, b_attn, W_proj, b_proj):
    x = np.asarray(x, np.float32)
    W_attn = np.asarray(W_attn, np.float32)
    b_attn = np.asarray(b_attn, np.float32)
    W_proj = np.asarray(W_proj, np.float32)
    b_proj = np.asarray(b_proj, np.float32)

    xT16 = np.ascontiguousarray(x.reshape(R, C).T).astype(np.float16)
    # tri[kk, j]: 1 if j >= kk (keep lower-triangular attention in S^T layout)
    tri = np.triu(np.ones((KA, KA), np.float16))

    Wq, Wk, Wv = (W_attn[:, i * C:(i + 1) * C] for i in range(3))
    bq, bk, bv_full = (b_attn[i * C:(i + 1) * C] for i in range(3))

    in_maps = []
    for core in range(NCORES):
        cols = slice(core * LC, (core + 1) * LC)
        wqk_l = np.concatenate([Wq[:, cols], Wk[:, cols]], axis=1)
        bqk_l = np.concatenate([bq[cols], bk[cols]])
        wv_l = Wv[:, cols].reshape(C, HL, HS)
        in_maps.append({
            "xT": xT16,
            "wqk": np.ascontiguousarray(wqk_l).astype(np.float16),
            "wv": np.ascontiguousarray(wv_l).astype(np.float16),
            "wp": np.ascontiguousarray(W_proj[cols, :]),
            "bqk": np.ascontiguousarray(bqk_l),
            "bv": np.ascontiguousarray(bv_full[cols]).astype(
                np.float16).reshape(1, LC),
            "trimask": tri,
        })
    return in_maps


def kernel(x, W_attn, b_attn, W_proj, b_proj):
    nc = _get_nc()
    in_maps = make_in_maps(x, W_attn, b_attn, W_proj, b_proj)
    res = run_bass_kernel_spmd(nc, in_maps, list(range(NCORES)))
    acc = res.results[0]["outT"].astype(np.float32)
    for corer in res.results[1:]:
        acc += corer["outT"].astype(np.float32)
    out = acc.T.reshape(B, T, C) + np.asarray(b_proj, np.float32)
    return out


# revision 13
# speedup vs baseline: 1.1525x; 1.1525x over previous
"""Causal self-attention (B=2, T=2048, C=1024, H=16) on 8 Trainium2 cores.

Sharding: tensor-parallel over heads (2 heads/core). Each core computes the
QKV projection for its heads, causal attention, and a partial c_proj output;
partials (and b_proj) are summed on the host.

v3 dataflow — natural-orientation AV (every matmul has a full 128-partition
output and near-minimal cost-model free size), with software pipelining
tuned for the in-order per-engine instruction streams:
  xT fp16 [C, B*T]                       (host pre-transposes x)
  qT/kT [128, B*T] fp16 = Wqk^T @ x + b  (PE, 512-free tiles)
  v_nat [k 128, 65] fp16 = x^T @ Wv | 1  (PE, 64-free tiles, bias via rank-1)
  S^T [k 128, q 512] f32 = K Q^T         (PE; causal tiles only, diag-trimmed)
  et = exp(S^T/8) fp16                   (ACT; tri-mask on diag via Pool)
  yp [q 128, 65] f32 += et_j^T @ v_nat   (PE; col 64 accumulates softmax sums)
  y = yp[:,0:64] * recip(yp[:,64]) f32r  (DVE per-partition scalars, no bcast)
  ynT [64, 128] = PE-transpose(y)        (f32r)
  out^T [c 128, rows] f32 = Wp^T @ ynT   (PE, f32r) -> fp16 staging -> DRAM

Scheduling: AV bursts are delayed 3 S-tiles behind their exp, transposes 2
bursts behind their normalize, and projection/c_proj work is drip-fed from a
work queue (one thunk per S-tile) so the in-order PE stream never waits on a
cross-engine semaphore chain. PSUM: 4 sps + 1 yp/transpose + 2 qk/cproj +
1 v = 8 banks exactly.
"""

from collections import deque

import numpy as np

import concourse.bass as bass
import concourse.tile as tile
from concourse import bacc, mybir
from concourse.bass_utils import run_bass_kernel_spmd
from concourse.masks import make_identity

F32 = mybir.dt.float32
F32R = mybir.dt.float32r
F16 = mybir.dt.float16

B, T, C, H = 2, 2048, 1024, 16
HS = C // H            # 64 head dim
NCORES = 8
HL = H // NCORES       # 2 local heads
LC = HL * HS           # 128 local q (or k, or v) channels
R = B * T              # 4096 rows
KC = C // 128          # 8 contraction chunks for projections
RT = 512               # row tile for qk-proj / c_proj
NRT = R // RT          # 8
QW = 512               # attention q window (S/exp tile width)
NQW = T // QW          # 4 per batch
KA = 128               # attention k chunk (partition dim)
NCC = C // 128         # 8 c_proj output chunks

AV_DELAY = 4           # S-tiles between exp issue and its AV burst
TP_DELAY = 2           # bursts between normalize and its PE transpose


def build_program():
    nc = bacc.Bacc("TRN2", target_bir_lowering=False, debug=False,
                   num_devices=NCORES)

    xT = nc.dram_tensor("xT", [C, R], F16, kind="ExternalInput").ap()
    wqk = nc.dram_tensor("wqk", [C, 2 * LC], F16, kind="ExternalInput").ap()
    wv = nc.dram_tensor("wv", [C, HL, HS], F16, kind="ExternalInput").ap()
    wp = nc.dram_tensor("wp", [LC, C], F32R, kind="ExternalInput").ap()
    bqk = nc.dram_tensor("bqk", [2 * LC], F32, kind="ExternalInput").ap()
    bv = nc.dram_tensor("bv", [1, LC], F16, kind="ExternalInput").ap()
    trimask = nc.dram_tensor("trimask", [KA, KA], F16, kind="ExternalInput").ap()
    outT = nc.dram_tensor("outT", [C, R], F16, kind="ExternalOutput").ap()

    with tile.TileContext(nc) as tc:
        with (
            tc.tile_pool(name="consts", bufs=1) as consts,
            tc.tile_pool(name="weights", bufs=1) as weights,
            tc.tile_pool(name="big", bufs=1) as big,
            tc.tile_pool(name="xs", bufs=3) as xs_pool,
            tc.tile_pool(name="et", bufs=26) as et_pool,
            tc.tile_pool(name="ysb", bufs=4) as ysb_pool,
            tc.tile_pool(name="rec", bufs=4) as rec_pool,
            tc.tile_pool(name="osb", bufs=2) as osb_pool,
            tc.tile_pool(name="sps", bufs=3, space="PSUM") as sps_pool,
            tc.tile_pool(name="ypt", bufs=2, space="PSUM") as ypt_pool,
            tc.tile_pool(name="qkps", bufs=2, space="PSUM") as qkps_pool,
            tc.tile_pool(name="vps", bufs=1, space="PSUM") as vps_pool,
        ):
            lp = nc.allow_low_precision(
                reason="fp16/f32r attention pipeline; ~1e-3 rel err validated")
            lp.__enter__()

            # ---- constants ----
            identity = consts.tile([128, 128], F32R)
            make_identity(nc, identity)
            ones1 = consts.tile([1, 128], F16)
            nc.vector.memset(ones1, 1.0)
            tri_sb = consts.tile([KA, KA], F16)
            bqk_sb = consts.tile([128, 2], F32)
            bv_sb = consts.tile([1, HL, HS], F16)

            # ---- weights ----
            wqk_sb = weights.tile([128, KC, 2 * LC], F16)
            wqk_r = wqk.rearrange("(kc p) n -> p kc n", p=128)
            wv_sb = weights.tile([128, KC, HL, HS], F16)
            wv_r = wv.rearrange("(kc p) h n -> p kc h n", p=128)
            wp_sb = weights.tile([LC, C], F32R)

            def load_consts():
                nc.sync.dma_start(out=tri_sb, in_=trimask)
                nc.sync.dma_start(
                    out=bqk_sb, in_=bqk.rearrange("(j p) -> p j", p=128))
                nc.sync.dma_start(out=bv_sb, in_=bv.rearrange("a (h n) -> a h n", h=HL))
                nc.sync.dma_start(out=wv_sb, in_=wv_r)
                nc.sync.dma_start(out=wp_sb, in_=wp)

            # ---- persistent activations ----
            qT_s = big.tile([LC, R], F16, tag="qT")
            kT_s = big.tile([LC, R], F16, tag="kT")
            ynT_s = big.tile([LC, R], F32R, tag="ynT")
            # v_nat[:, b, kci, h, 0:64] = v rows; col 64 = 1.0 (softmax sums)
            v_nat = big.tile([KA, B, T // KA, HL, HS + 1], F16, tag="vnat")


            def qkv_load(rt):
                x_sb = xs_pool.tile([128, KC, RT], F16, tag="xs",
                                    name=f"x_sb_rt{rt}")
                x_r = xT[:, rt * RT:(rt + 1) * RT].rearrange(
                    "(kc p) r -> p kc r", p=128)
                if rt == 0:
                    nc.sync.dma_start(out=x_sb[:, 0:1], in_=x_r[:, 0:1])
                    nc.sync.dma_start(out=x_sb[:, 1:KC], in_=x_r[:, 1:KC])
                else:
                    nc.sync.dma_start(out=x_sb, in_=x_r)
                return x_sb

            def qk_col(rt, x_sb, col, kc_outer_mate=None):
                """one projection column (q or k) for row tile rt."""
                span = slice(rt * RT, (rt + 1) * RT)
                dst = qT_s if col == 0 else kT_s
                ps = qkps_pool.tile([128, RT], F32, tag="qkps",
                                    name=f"qk_ps_rt{rt}c{col}")
                for kc in range(KC):
                    nc.tensor.matmul(
                        ps,
                        wqk_sb[:, kc, col * LC:(col + 1) * LC],
                        x_sb[:, kc, :],
                        start=(kc == 0), stop=(kc == KC - 1),
                    )
                nc.vector.tensor_scalar_add(
                    dst[:, span], ps, bqk_sb[:, col:col + 1])

            def v_chunk(rt, x_sb, ch):
                """v projection (natural layout, both heads) for one chunk."""
                b = (rt * RT) // T
                kci = (rt * RT - b * T) // KA + ch
                vp = vps_pool.tile([128, HL, HS], F32, tag="vps",
                                   name=f"vp_rt{rt}c{ch}")
                nc.tensor.matmul(vp, ones1, bv_sb, start=True, stop=False)
                for kc in range(KC):
                    nc.tensor.matmul(
                        vp,
                        x_sb[:, kc, ch * KA:(ch + 1) * KA],
                        wv_sb[:, kc, :, :],
                        start=False, stop=(kc == KC - 1),
                    )
                nc.vector.tensor_copy(v_nat[:, b, kci, :, 0:HS], vp)

            def qkv_thunks(rt):
                """work-queue thunks for one qkv row tile."""
                state = {}

                def load():
                    state["x"] = qkv_load(rt)

                return [
                    lambda: (load(), qk_col(rt, state["x"], 0)),
                    lambda: v_chunk(rt, state["x"], 0),
                    lambda: qk_col(rt, state["x"], 1),
                    lambda: v_chunk(rt, state["x"], 1),
                    lambda: v_chunk(rt, state["x"], 2),
                    lambda: v_chunk(rt, state["x"], 3),
                ]

            _ccsplit = [0]

            def cproj_thunks(rt, lo, hi, tail=False):
                """work-queue thunks for a c_proj row range (one per cc)."""
                w = hi - lo
                state = {}

                def get_osb():
                    if "o" not in state:
                        state["o"] = osb_pool.tile(
                            [128, NCC, RT], F16, tag="osb",
                            name=f"o_sb_rt{rt}_{lo}")
                    return state["o"]

                def cc_piece(cc):
                    o_sb = get_osb()
                    pps = qkps_pool.tile([128, RT], F32, tag="qkps",
                                         name=f"pps_rt{rt}_{lo}c{cc}")
                    nc.tensor.matmul(
                        pps[:, 0:w],
                        wp_sb[:, cc * 128:(cc + 1) * 128],
                        ynT_s[:, rt * RT + lo: rt * RT + hi],
                        start=True, stop=True,
                    )
                    if tail:
                        nc.vector.tensor_copy(
                            o_sb[:, cc, 0:w // 2], pps[:, 0:w // 2])
                        nc.gpsimd.tensor_copy(
                            o_sb[:, cc, w // 2:w], pps[:, w // 2:w])
                        return
                    # ~28/64 copies on DVE, rest on Pool (load balance)
                    s = _ccsplit[0]
                    _ccsplit[0] = (s + 1) % 16
                    if s % 16 < 7:
                        nc.vector.tensor_copy(o_sb[:, cc, 0:w], pps[:, 0:w])
                    else:
                        nc.gpsimd.tensor_copy(o_sb[:, cc, 0:w], pps[:, 0:w])

                def dma_out():
                    o_sb = get_osb()
                    nc.sync.dma_start(
                        out=outT.rearrange("(cc p) r -> p cc r", p=128)[
                            :, :, rt * RT + lo: rt * RT + hi],
                        in_=o_sb[:, :, 0:w],
                    )

                return [(lambda cc=cc: cc_piece(cc)) for cc in range(NCC)] + \
                    [dma_out]

            # ---- attention machinery ----
            gtile = [0]          # S tiles emitted
            nburst = [0]         # AV bursts fired
            pending_b = deque()  # (gtile at creation, fire closure)
            pending_t = deque()  # (nburst at creation, fire closure)
            def make_tp(b, h, j, y_sb, yt):
                def fire():
                    tdst = yt[0:HS, HS + 1:HS + 1 + 128].bitcast(F32R)
                    nc.tensor.transpose(tdst, y_sb, identity)
                    hsl = slice(h * HS, (h + 1) * HS)
                    base = b * T
                    dst = ynT_s[hsl, base + j * KA: base + (j + 1) * KA]
                    if j % 2 == 0:
                        nc.vector.tensor_copy(dst, tdst)
                    else:
                        nc.gpsimd.tensor_copy(dst, tdst)
                return fire

            def make_burst(b, h, q0, j, ets):
                def fire():
                    yt = ypt_pool.tile([128, HS + 1 + 128], F32, tag="ypt",
                                       name=f"ypt_b{b}h{h}j{j}")
                    yp = yt[:, 0:HS + 1]
                    jw = j * KA - q0
                    for k2 in range(j + 1):
                        et = ets[k2]
                        nc.tensor.matmul(
                            yp,
                            et[:, jw:jw + KA],
                            v_nat[:, b, k2, h, :],
                            start=(k2 == 0), stop=(k2 == j),
                        )
                    rec = rec_pool.tile([128, 1], F32, tag="rec",
                                        name=f"rec_b{b}h{h}j{j}")
                    nc.vector.reciprocal(rec, yp[:, HS:HS + 1])
                    y_sb = ysb_pool.tile([128, HS], F32R, tag="ysb",
                                         name=f"y_b{b}h{h}j{j}")
                    nc.vector.tensor_scalar_mul(y_sb, yp[:, 0:HS], rec)
                    pending_t.append((nburst[0], make_tp(b, h, j, y_sb, yt)))
                    nburst[0] += 1
                return fire

            def flush_pending(force_b=False, force_t=False):
                while True:
                    while pending_t and (force_t or
                                         nburst[0] - pending_t[0][0] >=
                                         TP_DELAY):
                        pending_t.popleft()[1]()
                    if pending_b and (force_b or
                                      gtile[0] - pending_b[0][0] >= AV_DELAY):
                        pending_b.popleft()[2]()
                        continue
                    break

            # ---- work queue ----
            workq = deque()  # (ready (widx,kc) | None, deadline widx | None, fn)

            def pop_work(widx, kc, budget=1):
                n = 0
                while workq and n < budget:
                    ready, _, fn = workq[0]
                    if ready is not None and (widx, kc) < ready:
                        break
                    workq.popleft()
                    fn()
                    n += 1

            def force_deadlines(widx):
                while workq:
                    _, deadline, fn = workq[0]
                    if deadline is None or deadline > widx:
                        break
                    workq.popleft()
                    fn()

            # ---- attention window ----
            def attn_window(widx, b, h, qw):
                base = b * T
                hsl = slice(h * HS, (h + 1) * HS)
                q0 = qw * QW
                nkc = (qw + 1) * (QW // KA)
                ets = []
                for kc in range(nkc):
                    diag = (kc * KA >= q0)
                    off = kc * KA - q0 if diag else 0
                    sps = sps_pool.tile([KA, QW], F32, tag="sps",
                                        name=f"sps_b{b}h{h}q{qw}k{kc}")
                    nc.tensor.matmul(
                        sps[:, off:QW],
                        kT_s[hsl, base + kc * KA: base + (kc + 1) * KA],
                        qT_s[hsl, base + q0 + off: base + q0 + QW],
                        start=True, stop=True,
                    )
                    gtile[0] += 1
                    et = et_pool.tile([KA, QW], F16, tag="et",
                                      name=f"et_b{b}h{h}q{qw}k{kc}")
                    nc.scalar.activation(
                        et[:, off:QW], sps[:, off:QW],
                        mybir.ActivationFunctionType.Exp,
                        scale=1.0 / np.sqrt(HS).item(),
                    )
                    if diag:
                        nc.gpsimd.tensor_mul(
                            et[:, off:off + KA], et[:, off:off + KA], tri_sb)
                        pending_b.append(
                            (gtile[0], kc, make_burst(b, h, q0, kc, ets)))
                    ets.append(et)
                    flush_pending()
                    pop_work(widx, kc)

            # ---- schedule ----
            x0 = qkv_load(0)
            nc.sync.dma_start(out=wqk_sb[:, 0:2], in_=wqk_r[:, 0:2])
            nc.sync.dma_start(out=wqk_sb[:, 2:KC], in_=wqk_r[:, 2:KC])
            load_consts()
            nc.vector.memset(v_nat[:, :, :, :, HS:HS + 1], 1.0)
            # rt0 inline, kc-outer-ish: q then k then v (x0 kc0 lands first)
            qk_col(0, x0, 0)
            v_chunk(0, x0, 0)
            qk_col(0, x0, 1)
            v_chunk(0, x0, 1)
            v_chunk(0, x0, 2)
            v_chunk(0, x0, 3)

            # queue the rest of the projection + c_proj work
            for rt in range(1, NRT):
                b = rt // NQW
                deadline = b * 8 + (rt % NQW)  # window (b, h0, qw=rt%4)
                for fn in qkv_thunks(rt):
                    workq.append((None, deadline, fn))
            for rt in range(NRT):
                b = rt // NQW
                qw = rt % NQW
                if rt == NRT - 1:
                    continue  # final row tile handled in the tail
                widx_done = b * 8 + 4 + qw  # window (b, h1, qw)
                ready = (widx_done + 1, 6)
                for fn in cproj_thunks(rt, 0, RT):
                    workq.append((ready, None, fn))

            windows = [(b, h, qw) for b in range(B) for h in range(HL)
                       for qw in range(NQW)]
            for widx, (b, h, qw) in enumerate(windows):
                force_deadlines(widx)
                attn_window(widx, b, h, qw)

            # ---- tail: drain pipeline + final row tile, fine-grained ----
            pop_work(10 ** 6, 0, budget=len(workq) + 1)
            # rows 3584:3840 need tps through j=13; rows 3840:4096 need j=15
            while pending_b and pending_b[0][1] <= 13:
                pending_b.popleft()[2]()
            flush_pending(force_t=True)
            for fn in cproj_thunks(NRT - 1, 0, 256):
                fn()
            while pending_b:
                pending_b.popleft()[2]()
            flush_pending(force_t=True)
            for fn in cproj_thunks(NRT - 1, 256, RT, tail=True):
                fn()

            lp.__exit__(None, None, None)

    nc.compile()
    return nc


_NC = None


def _get_nc():
    global _NC
    if _NC is None:
        _NC = build_program()
    return _NC


def make_in_maps(x, W_attn, b_attn, W_proj, b_proj):
    x = np.asarray(x, np.float32)
    W_attn = np.asarray(W_attn, np.float32)
    b_attn = np.asarray(b_attn, np.float32)
    W_proj = np.asarray(W_proj, np.float32)
    b_proj = np.asarray(b_proj, np.float32)

    xT16 = np.ascontiguousarray(x.reshape(R, C).T).astype(np.float16)
    # tri[kk, j]: 1 if j >= kk (keep lower-triangular attention in S^T layout)
    tri = np.triu(np.ones((KA, KA), np.float16))

    Wq, Wk, Wv = (W_attn[:, i * C:(i + 1) * C] for i in range(3))
    bq, bk, bv_full = (b_attn[i * C:(i + 1) * C] for i in range(3))

    in_maps = []
    for core in range(NCORES):
        cols = slice(core * LC, (core + 1) * LC)
        wqk_l = np.concatenate([Wq[:, cols], Wk[:, cols]], axis=1)
        bqk_l = np.concatenate([bq[cols], bk[cols]])
        wv_l = Wv[:, cols].reshape(C, HL, HS)
        in_maps.append({
            "xT": xT16,
            "wqk": np.ascontiguousarray(wqk_l).astype(np.float16),
            "wv": np.ascontiguousarray(wv_l).astype(np.float16),
            "wp": np.ascontiguousarray(W_proj[cols, :]),
            "bqk": np.ascontiguousarray(bqk_l),
            "bv": np.ascontiguousarray(bv_full[cols]).astype(
                np.float16).reshape(1, LC),
            "trimask": tri,
        })
    return in_maps


def kernel(x, W_attn, b_attn, W_proj, b_proj):
    nc = _get_nc()
    in_maps = make_in_maps(x, W_attn, b_attn, W_proj, b_proj)
    res = run_bass_kernel_spmd(nc, in_maps, list(range(NCORES)))
    acc = res.results[0]["outT"].astype(np.float32)
    for corer in res.results[1:]:
        acc += corer["outT"].astype(np.float32)
    out = acc.T.reshape(B, T, C) + np.asarray(b_proj, np.float32)
    return out


# revision 14
# speedup vs baseline: 1.1974x; 1.0390x over previous
"""Causal self-attention (B=2, T=2048, C=1024, H=16) on 8 Trainium2 cores.

Sharding: tensor-parallel over heads (2 heads/core). Each core computes the
QKV projection for its heads, causal attention, and a partial c_proj output;
partials (and b_proj) are summed on the host.

v3 dataflow — natural-orientation AV (every matmul has a full 128-partition
output and near-minimal cost-model free size), with software pipelining
tuned for the in-order per-engine instruction streams:
  xT fp16 [C, B*T]                       (host pre-transposes x)
  qT/kT [128, B*T] fp16 = Wqk^T @ x + b  (PE, 512-free tiles)
  v_nat [k 128, 65] fp16 = x^T @ Wv | 1  (PE, 64-free tiles, bias via rank-1)
  S^T [k 128, q 512] f32 = K Q^T         (PE; causal tiles only, diag-trimmed)
  et = exp(S^T/8) fp16                   (ACT; tri-mask on diag via Pool)
  yp [q 128, 65] f32 += et_j^T @ v_nat   (PE; col 64 accumulates softmax sums)
  y = yp[:,0:64] * recip(yp[:,64]) f32r  (DVE per-partition scalars, no bcast)
  ynT [64, 128] = PE-transpose(y)        (f32r)
  out^T [c 128, rows] f32 = Wp^T @ ynT   (PE, f32r) -> fp16 staging -> DRAM

Scheduling: AV bursts are delayed 3 S-tiles behind their exp, transposes 2
bursts behind their normalize, and projection/c_proj work is drip-fed from a
work queue (one thunk per S-tile) so the in-order PE stream never waits on a
cross-engine semaphore chain. PSUM: 4 sps + 1 yp/transpose + 2 qk/cproj +
1 v = 8 banks exactly.
"""

from collections import deque

import numpy as np

import concourse.bass as bass
import concourse.tile as tile
from concourse import bacc, mybir
from concourse.bass_utils import run_bass_kernel_spmd
from concourse.masks import make_identity

F32 = mybir.dt.float32
F32R = mybir.dt.float32r
F16 = mybir.dt.float16

B, T, C, H = 2, 2048, 1024, 16
HS = C // H            # 64 head dim
NCORES = 8
HL = H // NCORES       # 2 local heads
LC = HL * HS           # 128 local q (or k, or v) channels
R = B * T              # 4096 rows
KC = C // 128          # 8 contraction chunks for projections
RT = 512               # row tile for qk-proj / c_proj
NRT = R // RT          # 8
QW = 512               # attention q window (S/exp tile width)
NQW = T // QW          # 4 per batch
KA = 128               # attention k chunk (partition dim)
NCC = C // 128         # 8 c_proj output chunks

AV_DELAY = 4           # S-tiles between exp issue and its AV burst
TP_DELAY = 2           # bursts between normalize and its PE transpose


def build_program():
    nc = bacc.Bacc("TRN2", target_bir_lowering=False, debug=False,
                   num_devices=NCORES)

    xT = nc.dram_tensor("xT", [C, R], F16, kind="ExternalInput").ap()
    wqk = nc.dram_tensor("wqk", [C, 2 * LC], F16, kind="ExternalInput").ap()
    wv = nc.dram_tensor("wv", [C, HL, HS], F16, kind="ExternalInput").ap()
    wp = nc.dram_tensor("wp", [LC, C], F32R, kind="ExternalInput").ap()
    bqk = nc.dram_tensor("bqk", [2 * LC], F32, kind="ExternalInput").ap()
    bv = nc.dram_tensor("bv", [1, LC], F16, kind="ExternalInput").ap()
    trimask = nc.dram_tensor("trimask", [KA, KA], F16, kind="ExternalInput").ap()
    outT = nc.dram_tensor("outT", [C, R], F16, kind="ExternalOutput").ap()

    with tile.TileContext(nc) as tc:
        with (
            tc.tile_pool(name="consts", bufs=1) as consts,
            tc.tile_pool(name="weights", bufs=1) as weights,
            tc.tile_pool(name="big", bufs=1) as big,
            tc.tile_pool(name="xs", bufs=3) as xs_pool,
            tc.tile_pool(name="et", bufs=26) as et_pool,
            tc.tile_pool(name="ysb", bufs=4) as ysb_pool,
            tc.tile_pool(name="rec", bufs=4) as rec_pool,
            tc.tile_pool(name="osb", bufs=2) as osb_pool,
            tc.tile_pool(name="sps", bufs=3, space="PSUM") as sps_pool,
            tc.tile_pool(name="ypt", bufs=2, space="PSUM") as ypt_pool,
            tc.tile_pool(name="qkps", bufs=2, space="PSUM") as qkps_pool,
            tc.tile_pool(name="vps", bufs=1, space="PSUM") as vps_pool,
        ):
            lp = nc.allow_low_precision(
                reason="fp16/f32r attention pipeline; ~1e-3 rel err validated")
            lp.__enter__()

            # ---- constants ----
            identity = consts.tile([128, 128], F32R)
            make_identity(nc, identity)
            ones1 = consts.tile([1, 128], F16)
            nc.vector.memset(ones1, 1.0)
            tri_sb = consts.tile([KA, KA], F16)
            bqk_sb = consts.tile([128, 2], F32)
            bv_sb = consts.tile([1, HL, HS], F16)

            # ---- weights ----
            wqk_sb = weights.tile([128, KC, 2 * LC], F16)
            wqk_r = wqk.rearrange("(kc p) n -> p kc n", p=128)
            wv_sb = weights.tile([128, KC, HL, HS], F16)
            wv_r = wv.rearrange("(kc p) h n -> p kc h n", p=128)
            wp_sb = weights.tile([LC, C], F32R)

            def load_consts():
                nc.sync.dma_start(out=tri_sb, in_=trimask)
                nc.sync.dma_start(
                    out=bqk_sb, in_=bqk.rearrange("(j p) -> p j", p=128))
                nc.sync.dma_start(out=bv_sb, in_=bv.rearrange("a (h n) -> a h n", h=HL))
                nc.sync.dma_start(out=wv_sb, in_=wv_r)
                nc.sync.dma_start(out=wp_sb, in_=wp)

            # ---- persistent activations ----
            qT_s = big.tile([LC, R], F16, tag="qT")
            kT_s = big.tile([LC, R], F16, tag="kT")
            ynT_s = big.tile([LC, R], F32R, tag="ynT")
            # v_nat[:, b, kci, h, 0:64] = v rows; col 64 = 1.0 (softmax sums)
            v_nat = big.tile([KA, B, T // KA, HL, HS + 1], F16, tag="vnat")


            def qkv_load(rt):
                x_sb = xs_pool.tile([128, KC, RT], F16, tag="xs",
                                    name=f"x_sb_rt{rt}")
                x_r = xT[:, rt * RT:(rt + 1) * RT].rearrange(
                    "(kc p) r -> p kc r", p=128)
                if rt == 0:
                    nc.sync.dma_start(out=x_sb[:, 0:1], in_=x_r[:, 0:1])
                    nc.sync.dma_start(out=x_sb[:, 1:KC], in_=x_r[:, 1:KC])
                else:
                    nc.sync.dma_start(out=x_sb, in_=x_r)
                return x_sb

            def qk_col(rt, x_sb, col, kc_outer_mate=None):
                """one projection column (q or k) for row tile rt."""
                span = slice(rt * RT, (rt + 1) * RT)
                dst = qT_s if col == 0 else kT_s
                ps = qkps_pool.tile([128, RT], F32, tag="qkps",
                                    name=f"qk_ps_rt{rt}c{col}")
                for kc in range(KC):
                    nc.tensor.matmul(
                        ps,
                        wqk_sb[:, kc, col * LC:(col + 1) * LC],
                        x_sb[:, kc, :],
                        start=(kc == 0), stop=(kc == KC - 1),
                    )
                nc.vector.tensor_scalar_add(
                    dst[:, span], ps, bqk_sb[:, col:col + 1])

            def v_chunk(rt, x_sb, ch):
                """v projection (natural layout, both heads) for one chunk."""
                b = (rt * RT) // T
                kci = (rt * RT - b * T) // KA + ch
                vp = vps_pool.tile([128, HL, HS], F32, tag="vps",
                                   name=f"vp_rt{rt}c{ch}")
                nc.tensor.matmul(vp, ones1, bv_sb, start=True, stop=False)
                for kc in range(KC):
                    nc.tensor.matmul(
                        vp,
                        x_sb[:, kc, ch * KA:(ch + 1) * KA],
                        wv_sb[:, kc, :, :],
                        start=False, stop=(kc == KC - 1),
                    )
                nc.gpsimd.tensor_copy(v_nat[:, b, kci, :, 0:HS], vp)

            def qkv_thunks(rt):
                """work-queue thunks for one qkv row tile."""
                state = {}

                def load():
                    state["x"] = qkv_load(rt)

                return [
                    lambda: (load(), qk_col(rt, state["x"], 0)),
                    lambda: v_chunk(rt, state["x"], 0),
                    lambda: qk_col(rt, state["x"], 1),
                    lambda: v_chunk(rt, state["x"], 1),
                    lambda: v_chunk(rt, state["x"], 2),
                    lambda: v_chunk(rt, state["x"], 3),
                ]

            _ccsplit = [0]

            def cproj_thunks(rt, lo, hi, tail=False):
                """work-queue thunks for a c_proj row range (one per cc)."""
                w = hi - lo
                state = {}

                def get_osb():
                    if "o" not in state:
                        state["o"] = osb_pool.tile(
                            [128, NCC, RT], F16, tag="osb",
                            name=f"o_sb_rt{rt}_{lo}")
                    return state["o"]

                def cc_piece(cc):
                    o_sb = get_osb()
                    pps = qkps_pool.tile([128, RT], F32, tag="qkps",
                                         name=f"pps_rt{rt}_{lo}c{cc}")
                    nc.tensor.matmul(
                        pps[:, 0:w],
                        wp_sb[:, cc * 128:(cc + 1) * 128],
                        ynT_s[:, rt * RT + lo: rt * RT + hi],
                        start=True, stop=True,
                    )
                    if tail:
                        nc.vector.tensor_copy(
                            o_sb[:, cc, 0:w // 2], pps[:, 0:w // 2])
                        nc.gpsimd.tensor_copy(
                            o_sb[:, cc, w // 2:w], pps[:, w // 2:w])
                        return
                    # ~28/64 copies on DVE, rest on Pool (load balance)
                    s = _ccsplit[0]
                    _ccsplit[0] = (s + 1) % 16
                    if s % 16 < 6:
                        nc.vector.tensor_copy(o_sb[:, cc, 0:w], pps[:, 0:w])
                    else:
                        nc.gpsimd.tensor_copy(o_sb[:, cc, 0:w], pps[:, 0:w])

                def dma_out():
                    o_sb = get_osb()
                    nc.sync.dma_start(
                        out=outT.rearrange("(cc p) r -> p cc r", p=128)[
                            :, :, rt * RT + lo: rt * RT + hi],
                        in_=o_sb[:, :, 0:w],
                    )

                return [(lambda cc=cc: cc_piece(cc)) for cc in range(NCC)] + \
                    [dma_out]

            # ---- attention machinery ----
            gtile = [0]          # S tiles emitted
            nburst = [0]         # AV bursts fired
            pending_b = deque()  # (gtile at creation, fire closure)
            pending_t = deque()  # (nburst at creation, fire closure)
            def make_tp(b, h, j, y_sb, yt):
                def fire():
                    tdst = yt[0:HS, HS + 1:HS + 1 + 128].bitcast(F32R)
                    nc.tensor.transpose(tdst, y_sb, identity)
                    hsl = slice(h * HS, (h + 1) * HS)
                    base = b * T
                    dst = ynT_s[hsl, base + j * KA: base + (j + 1) * KA]
                    if j % 2 == 0:
                        nc.vector.tensor_copy(dst, tdst)
                    else:
                        nc.gpsimd.tensor_copy(dst, tdst)
                return fire

            def make_burst(b, h, q0, j, ets):
                def fire():
                    yt = ypt_pool.tile([128, HS + 1 + 128], F32, tag="ypt",
                                       name=f"ypt_b{b}h{h}j{j}")
                    yp = yt[:, 0:HS + 1]
                    jw = j * KA - q0
                    for k2 in range(j + 1):
                        et = ets[k2]
                        nc.tensor.matmul(
                            yp,
                            et[:, jw:jw + KA],
                            v_nat[:, b, k2, h, :],
                            start=(k2 == 0), stop=(k2 == j),
                        )
                    rec = rec_pool.tile([128, 1], F32, tag="rec",
                                        name=f"rec_b{b}h{h}j{j}")
                    nc.vector.reciprocal(rec, yp[:, HS:HS + 1])
                    y_sb = ysb_pool.tile([128, HS], F32R, tag="ysb",
                                         name=f"y_b{b}h{h}j{j}")
                    nc.vector.tensor_scalar_mul(y_sb, yp[:, 0:HS], rec)
                    pending_t.append((nburst[0], make_tp(b, h, j, y_sb, yt)))
                    nburst[0] += 1
                return fire

            def flush_pending(force_b=False, force_t=False):
                while True:
                    while pending_t and (force_t or
                                         nburst[0] - pending_t[0][0] >=
                                         TP_DELAY):
                        pending_t.popleft()[1]()
                    if pending_b and (force_b or
                                      gtile[0] - pending_b[0][0] >= AV_DELAY):
                        pending_b.popleft()[2]()
                        continue
                    break

            # ---- work queue ----
            workq = deque()  # (ready (widx,kc) | None, deadline widx | None, fn)

            def pop_work(widx, kc, budget=1):
                n = 0
                while workq and n < budget:
                    ready, _, fn = workq[0]
                    if ready is not None and (widx, kc) < ready:
                        break
                    workq.popleft()
                    fn()
                    n += 1

            def force_deadlines(widx):
                while workq:
                    _, deadline, fn = workq[0]
                    if deadline is None or deadline > widx:
                        break
                    workq.popleft()
                    fn()

            # ---- attention window ----
            def attn_window(widx, b, h, qw):
                base = b * T
                hsl = slice(h * HS, (h + 1) * HS)
                q0 = qw * QW
                nkc = (qw + 1) * (QW // KA)
                ets = []
                for kc in range(nkc):
                    diag = (kc * KA >= q0)
                    off = kc * KA - q0 if diag else 0
                    sps = sps_pool.tile([KA, QW], F32, tag="sps",
                                        name=f"sps_b{b}h{h}q{qw}k{kc}")
                    nc.tensor.matmul(
                        sps[:, off:QW],
                        kT_s[hsl, base + kc * KA: base + (kc + 1) * KA],
                        qT_s[hsl, base + q0 + off: base + q0 + QW],
                        start=True, stop=True,
                    )
                    gtile[0] += 1
                    et = et_pool.tile([KA, QW], F16, tag="et",
                                      name=f"et_b{b}h{h}q{qw}k{kc}")
                    nc.scalar.activation(
                        et[:, off:QW], sps[:, off:QW],
                        mybir.ActivationFunctionType.Exp,
                        scale=1.0 / np.sqrt(HS).item(),
                    )
                    if diag:
                        nc.vector.tensor_tensor(
                            et[:, off:off + KA], et[:, off:off + KA], tri_sb,
                            op=mybir.AluOpType.mult)
                        pending_b.append(
                            (gtile[0], kc, make_burst(b, h, q0, kc, ets)))
                    ets.append(et)
                    flush_pending()
                    pop_work(widx, kc)

            # ---- schedule ----
            nc.sync.dma_start(out=wqk_sb[:, 0:2], in_=wqk_r[:, 0:2])
            x0 = qkv_load(0)
            nc.sync.dma_start(out=wqk_sb[:, 2:KC], in_=wqk_r[:, 2:KC])
            load_consts()
            nc.vector.memset(v_nat[:, :, :, :, HS:HS + 1], 1.0)
            # rt0 inline, kc-outer-ish: q then k then v (x0 kc0 lands first)
            qk_col(0, x0, 0)
            v_chunk(0, x0, 0)
            qk_col(0, x0, 1)
            v_chunk(0, x0, 1)
            v_chunk(0, x0, 2)
            v_chunk(0, x0, 3)

            # queue the rest of the projection + c_proj work
            for rt in range(1, NRT):
                b = rt // NQW
                deadline = b * 8 + (rt % NQW)  # window (b, h0, qw=rt%4)
                for fn in qkv_thunks(rt):
                    workq.append((None, deadline, fn))
            for rt in range(NRT):
                b = rt // NQW
                qw = rt % NQW
                if rt == NRT - 1:
                    continue  # final row tile handled in the tail
                widx_done = b * 8 + 4 + qw  # window (b, h1, qw)
                ready = (widx_done + 1, 6)
                for fn in cproj_thunks(rt, 0, RT):
                    workq.append((ready, None, fn))

            windows = [(b, h, qw) for b in range(B) for h in range(HL)
                       for qw in range(NQW)]
            for widx, (b, h, qw) in enumerate(windows):
                force_deadlines(widx)
                attn_window(widx, b, h, qw)

            # ---- tail: drain pipeline + final row tile, fine-grained ----
            pop_work(10 ** 6, 0, budget=len(workq) + 1)
            # rows 3584:3840 need tps through j=13; rows 3840:4096 need j=15
            while pending_b and pending_b[0][1] <= 13:
                pending_b.popleft()[2]()
            flush_pending(force_t=True)
            for fn in cproj_thunks(NRT - 1, 0, 256):
                fn()
            while pending_b:
                pending_b.popleft()[2]()
            flush_pending(force_t=True)
            for fn in cproj_thunks(NRT - 1, 256, RT, tail=True):
                fn()

            lp.__exit__(None, None, None)

    nc.compile()
    return nc


_NC = None


def _get_nc():
    global _NC
    if _NC is None:
        _NC = build_program()
    return _NC


def make_in_maps(x, W_attn, b_attn, W_proj, b_proj):
    x = np.asarray(x, np.float32)
    W_attn = np.asarray(W_attn, np.float32)
    b_attn = np.asarray(b_attn, np.float32)
    W_proj = np.asarray(W_proj, np.float32)
    b_proj = np.asarray(b_proj, np.float32)

    xT16 = np.ascontiguousarray(x.reshape(R, C).T).astype(np.float16)
    # tri[kk, j]: 1 if j >= kk (keep lower-triangular attention in S^T layout)
    tri = np.triu(np.ones((KA, KA), np.float16))

    Wq, Wk, Wv = (W_attn[:, i * C:(i + 1) * C] for i in range(3))
    bq, bk, bv_full = (b_attn[i * C:(i + 1) * C] for i in range(3))

    in_maps = []
    for core in range(NCORES):
        cols = slice(core * LC, (core + 1) * LC)
        wqk_l = np.concatenate([Wq[:, cols], Wk[:, cols]], axis=1)
        bqk_l = np.concatenate([bq[cols], bk[cols]])
        wv_l = Wv[:, cols].reshape(C, HL, HS)
        in_maps.append({
            "xT": xT16,
            "wqk": np.ascontiguousarray(wqk_l).astype(np.float16),
            "wv": np.ascontiguousarray(wv_l).astype(np.float16),
            "wp": np.ascontiguousarray(W_proj[cols, :]),
            "bqk": np.ascontiguousarray(bqk_l),
            "bv": np.ascontiguousarray(bv_full[cols]).astype(
                np.float16).reshape(1, LC),
            "trimask": tri,
        })
    return in_maps


def kernel(x, W_attn, b_attn, W_proj, b_proj):
    nc = _get_nc()
    in_maps = make_in_maps(x, W_attn, b_attn, W_proj, b_proj)
    res = run_bass_kernel_spmd(nc, in_maps, list(range(NCORES)))
    acc = res.results[0]["outT"].astype(np.float32)
    for corer in res.results[1:]:
        acc += corer["outT"].astype(np.float32)
    out = acc.T.reshape(B, T, C) + np.asarray(b_proj, np.float32)
    return out
